# revision 4
# baseline (speedup 1.0000x reference)
"""Distributed inverse real vector SHT on 8 Trainium2 NeuronCores.

Decomposition (2D polar x azimuth, per the original model's parallelism):
  Stage 1 (sharded over m): for each m, the four Legendre contractions are
  two accumulating matmuls  Z[m] = X1[m]^T @ dT0[m] + X2[m]^T @ dT1[m]
  where the 128 columns of X1/X2 pack the four (re/im x s/t) input blocks
  with signs arranged so the PSUM accumulation directly produces
  rows [srl, sim, tim, trl] (no separate combine step).
  All-to-all: re-shard from m-split to nlat(k)-split.
  Stage 2 (sharded over k): the irfft is a real matmul against precomputed
  cos/sin tables contracting over (re/im, m).
"""
import sys
import os
sys.path.insert(0, '/opt/trn_rl_repo')
import numpy as np
import ml_dtypes

from concourse import bacc, tile, mybir
from concourse.bass_utils import run_bass_kernel_spmd

B, C, L, M, K, N = 1, 32, 361, 361, 361, 720
NC = 8
MP = 368                    # m padded to 8*46
MC = MP // NC               # 46 m's per core
KC = 46                     # real k's per core
KG = 48                     # padded per-core k width (12 groups of 4)
KPP = NC * KG               # 384: padded k total in stage-1 tables
LP = 384                    # l padded to 3*128
LCH = 3
NG = KG // 4                # 12 kj-groups per core
BF16 = ml_dtypes.bfloat16

_CACHE = {}


def _build():
    nc = bacc.Bacc("TRN2", target_bir_lowering=False, debug=False,
                   num_devices=NC)
    xsh = nc.dram_tensor("xsh", [128, MC, 2, LCH, 128], mybir.dt.bfloat16,
                         kind="ExternalInput")
    dsh = nc.dram_tensor("dsh", [128, MC, 2, LCH, KPP], mybir.dt.bfloat16,
                         kind="ExternalInput")
    ctab = nc.dram_tensor("ctab", [128, 2, LCH, N], mybir.dt.bfloat16,
                          kind="ExternalInput")
    outsh = nc.dram_tensor("outsh", [2, C, KG, N], mybir.dt.float32,
                           kind="ExternalOutput")

    m_blocks = []
    m0 = 0
    while m0 < MC:
        cnt = min(8, MC - m0)
        m_blocks.append((m0, cnt))
        m0 += cnt

    with tile.TileContext(nc) as tc:
        with tc.tile_pool(name="dram", bufs=1, space="DRAM") as dram:
            a2a_in = dram.tile([NC, MC, 128, KG], mybir.dt.bfloat16)
            a2a_out = dram.tile([NC, MC, 128, KG], mybir.dt.bfloat16)

            # ---------------- stage 1: Legendre contractions (m-sharded)
            with tc.tile_pool(name="s1", bufs=2) as s1, \
                 tc.tile_pool(name="zs", bufs=1) as zs, \
                 tc.tile_pool(name="ps1", bufs=4, space="PSUM") as ps1:
                zstage = zs.tile([128, MC, KPP], mybir.dt.bfloat16)
                for (m0, cnt) in m_blocks:
                    dt = s1.tile([128, 8, 2, LCH, KPP], mybir.dt.bfloat16,
                                 tag="dt")
                    xt = s1.tile([128, 8, 2, LCH, 128], mybir.dt.bfloat16,
                                 tag="xt")
                    nc.sync.dma_start(out=dt[:, :cnt], in_=dsh[:, m0:m0+cnt])
                    nc.sync.dma_start(out=xt[:, :cnt], in_=xsh[:, m0:m0+cnt])
                    for ml in range(cnt):
                        zt = ps1.tile([128, KPP], mybir.dt.float32, tag="zt")
                        for lc in range(LCH):
                            for w in range(2):
                                nc.tensor.matmul(
                                    out=zt[:],
                                    lhsT=xt[:, ml, w, lc, :],
                                    rhs=dt[:, ml, w, lc, :],
                                    start=(lc == 0 and w == 0),
                                    stop=(lc == LCH - 1 and w == 1),
                                )
                        nc.vector.tensor_copy(out=zstage[:, m0 + ml, :],
                                              in_=zt[:])
                # scatter into a2a blocks: dest [m, c2, kj] <- src [c2, m, kj]
                for kg in range(NC):
                    nc.sync.dma_start(
                        out=a2a_in[kg].rearrange("m c k -> c m k"),
                        in_=zstage[:, :, kg*KG:(kg+1)*KG],
                    )

            nc.gpsimd.collective_compute(
                "AllToAll", mybir.AluOpType.bypass,
                replica_groups=[list(range(NC))],
                ins=[a2a_in.opt()], outs=[a2a_out.opt()],
            )

            # ---------------- stage 2: irfft as matmul (k-sharded)
            # a2a_out viewed as [MP, 128, KG]: m-contiguous.
            with tc.tile_pool(name="s2", bufs=1) as s2, \
                 tc.tile_pool(name="s2r", bufs=2) as s2r, \
                 tc.tile_pool(name="ob", bufs=3) as ob, \
                 tc.tile_pool(name="ps2", bufs=4, space="PSUM") as ps2:
                ct = s2.tile([128, 2, LCH, N], mybir.dt.bfloat16, tag="ct")
                nc.sync.dma_start(out=ct[:], in_=ctab[:])
                mchunks = [(0, 128), (128, 128), (256, MP - 256)]
                zt2 = []
                for mc, (mm0, mcnt) in enumerate(mchunks):
                    raw = s2r.tile([128, 128, KG], mybir.dt.bfloat16,
                                   tag="zt2raw")
                    nc.sync.dma_start(
                        out=raw[:mcnt],
                        in_=a2a_out.rearrange("g m c k -> (g m) c k")[mm0:mm0+mcnt],
                    )
                    # repack so each (c2-block, kj-group) is a contiguous
                    # 128-wide stationary operand: [m, b, g, c, kj]
                    t = s2.tile([128, 4, NG, 32, 4], mybir.dt.bfloat16,
                                tag=f"zt2_{mc}")
                    for b in range(4):
                        nc.vector.tensor_copy(
                            out=t[:mcnt, b],
                            in_=raw[:mcnt, b*32:(b+1)*32, :].rearrange(
                                "m c (g k) -> m g c k", k=4),
                        )
                    zt2.append(t)

                # comp 0 (s): srl rows (b=0) w/ Cre, sim rows (b=1) w/ Cim
                # comp 1 (t): trl rows (b=3) w/ Cre, tim rows (b=2) w/ Cim
                comp_seq = [((0, 0), (1, 1)), ((3, 0), (2, 1))]
                for comp in range(2):
                    for g in range(NG):
                        po0 = ps2.tile([128, 360], mybir.dt.float32, tag="po0")
                        po1 = ps2.tile([128, 360], mybir.dt.float32, tag="po1")
                        i = 0
                        for (b, reim) in comp_seq[comp]:
                            for mc, (mm0, mcnt) in enumerate(mchunks):
                                lhsT = zt2[mc][:mcnt, b, g]
                                nc.tensor.matmul(
                                    out=po0[:], lhsT=lhsT,
                                    rhs=ct[:mcnt, reim, mc, 0:360],
                                    start=(i == 0), stop=(i == 5))
                                nc.tensor.matmul(
                                    out=po1[:], lhsT=lhsT,
                                    rhs=ct[:mcnt, reim, mc, 360:720],
                                    start=(i == 0), stop=(i == 5))
                                i += 1
                        osb = ob.tile([128, N], mybir.dt.float32, tag="osb")
                        nc.vector.tensor_copy(out=osb[:, 0:360], in_=po0[:])
                        nc.vector.tensor_copy(out=osb[:, 360:720], in_=po1[:])
                        nc.sync.dma_start(
                            out=outsh[comp, :, g*4:(g+1)*4, :],
                            in_=osb[:],
                        )
    nc.compile()
    return nc


def _host_prep(x_re, x_im, d0, d1):
    xr0, xr1 = x_re[0, :, 0], x_re[0, :, 1]   # (32, L, M)
    xi0, xi1 = x_im[0, :, 0], x_im[0, :, 1]

    def mkx(blocks):
        x = np.concatenate(blocks, axis=0)            # (128, L, M)
        x = np.transpose(x, (2, 1, 0))                # (M, L, 128)
        xp = np.zeros((MP, LP, 128), BF16)
        xp[:M, :L] = x
        return xp
    X1 = mkx([xr0, xi0, -xi1, -xr1])
    X2 = mkx([-xi1, xr1, xr0, -xi0])
    # xsh[core][p, ml, which, lc, c'] = X{which}[core*MC+ml, lc*128+p, c']
    xv = np.stack([X1, X2], axis=1)                   # (MP, 2, LP, 128)
    xv = xv.reshape(NC, MC, 2, LCH, 128, 128)         # (i, ml, w, lc, p, c)
    xv = np.ascontiguousarray(xv.transpose(0, 4, 1, 2, 3, 5))

    def mkd(d):
        dp = np.zeros((MP, LP, KPP), BF16)
        dt = np.transpose(d, (0, 2, 1))               # (M, L, K)
        for kg in range(NC):
            k0 = kg * KC
            k1 = min(K, k0 + KC)
            dp[:M, :L, kg*KG:kg*KG + (k1-k0)] = dt[:, :, k0:k1]
        return dp
    D0, D1 = mkd(d0), mkd(d1)
    dv = np.stack([D0, D1], axis=1)                   # (MP, 2, LP, KPP)
    dv = dv.reshape(NC, MC, 2, LCH, 128, KPP)
    dv = np.ascontiguousarray(dv.transpose(0, 4, 1, 2, 3, 5))

    m = np.arange(MP, dtype=np.float64)[:, None]
    n = np.arange(N, dtype=np.float64)[None, :]
    th = 2.0 * np.pi * (m * n) / N
    w = np.full((MP, 1), 2.0); w[0] = 1.0; w[360] = 1.0; w[361:] = 0.0
    Cre = (w * np.cos(th)).astype(np.float32)
    Cim = (-w * np.sin(th)).astype(np.float32)
    Cim[0] = 0.0; Cim[360] = 0.0; Cim[361:] = 0.0
    cv = np.stack([Cre, Cim], axis=1)                 # (MP, 2, N)
    cv = np.concatenate(
        [cv, np.zeros((LCH * 128 - MP, 2, N), np.float32)], axis=0)
    cv = cv.reshape(LCH, 128, 2, N)
    cv = np.ascontiguousarray(cv.transpose(1, 2, 0, 3)).astype(BF16)
    return xv, dv, cv


def kernel(x_re, x_im, d0, d1):
    if "nc" not in _CACHE:
        _CACHE["nc"] = _build()
    nc = _CACHE["nc"]

    xv, dv, cv = _host_prep(np.asarray(x_re), np.asarray(x_im),
                            np.asarray(d0), np.asarray(d1))
    in_maps = [{"xsh": xv[i], "dsh": dv[i], "ctab": cv} for i in range(NC)]
    res = run_bass_kernel_spmd(nc, in_maps, list(range(NC)))

    out = np.empty((B, C, 2, K, N), np.float32)
    for i in range(NC):
        k0 = i * KC
        k1 = min(K, k0 + KC)
        o = res.results[i]["outsh"]        # [2, C, KG, N]
        out[0, :, 0, k0:k1] = o[0, :, :k1-k0]
        out[0, :, 1, k0:k1] = o[1, :, :k1-k0]
    return out
